# revision 23
# baseline (speedup 1.0000x reference)
"""Multihead attention (B=2, L=2048, D=1024, 16 heads) on 8 trn2 cores.

Sharding: tensor-parallel over heads — 2 heads per core. Each core computes
q/k/v projections for its 128 columns of Wq/Wk/Wv, full attention for its two
heads, and a partial output projection against its 128 rows of Wo. The host
sums the 8 bf16 partials and adds bo.

Compute is bf16 on the PE with fp32 PSUM accumulation.

Per-core layouts (host-side transpose+cast of x; weights pre-shuffled to
[partition, k-tile, col] so their DMAs are clean 1KB-segment 2D transfers):
  qT/kT: [128(d_local), B*L]     — contraction-major for the scoresT matmuls
  v:     computed as vT [d, s] (full-width N=512 streams, not the
         LDWEIGHTS-bound N=128 natural-layout form), bv folded in, then
         PE-transposed into vaug [s, 130] = per head [64 v dims | ones col]
         so attn@v emits the softmax denominator as row 64
  scoresT[s, l] per (b, l-chunk), both heads in one 2-bank PSUM tile so one
  ScalarE exp covers them (no max subtraction: scores ~ N(0,1) for this
  model), attn@v accumulated over s-tiles in PSUM, normalized via gpsimd
  partition_broadcast + DVE fast reciprocal.

Scheduling: a single flat software pipeline over all (chunk, s-tile) units —
the attn@v stream globally trails the scores/exp stream by GLOBAL_LAG units
across chunk boundaries, and the first av of each chunk is held until st 8 so
the previous chunk's PSUM evacuation is never on the PE critical path. Only
projection chunk 0 runs before attention; chunks 1..7 are staggered in as PE
filler. x tiles load as one 3D-AP DMA per chunk (descriptor-gen is ~700ns a
pop and otherwise gates startup; the first chunk is split in 2-k-tile pieces
across three queues so the first matmul only waits ~256KB). O-projection runs
in 128-row strips on odd s-tiles of each chunk's second half; the final
chunk's normalize+o-proj is strip-pipelined with a PE-matmul denominator
broadcast straight off the avs row-64 slice and ScalarE output evacuation
(both engines are otherwise idle in the tail).
"""

from contextlib import ExitStack

import ml_dtypes
import numpy as np

import concourse.bacc as bacc
import concourse.mybir as mybir
import concourse.tile as tile
from concourse import masks
from concourse.bass_utils import run_bass_kernel_spmd

D_MODEL = 1024
N_HEAD = 16
HEAD_DIM = 64
B = 2
L = 2048
N_CORES = 8
HPC = N_HEAD // N_CORES  # heads per core
MLOC = HPC * HEAD_DIM  # 128: local d width per core

F32 = mybir.dt.float32
BF16 = mybir.dt.bfloat16
NPBF16 = ml_dtypes.bfloat16


def build_nc(Lb=L, lc_size=512, nch=512):
    """Build the per-core Bass program. Lb = sequence length per batch."""
    BLb = B * Lb
    KT = D_MODEL // 128  # 8 contraction tiles for the projections
    n_nch = BLb // nch  # projection column chunks
    st_per_nch = nch // 128  # s-tiles per projection chunk
    n_lc = Lb // lc_size  # attention l-chunks per batch
    n_st = Lb // 128  # s-tiles per batch
    n_lt = lc_size // 128  # l-tiles (128) per l-chunk

    nc = bacc.Bacc("TRN2", target_bir_lowering=False, debug=False)

    xT = nc.dram_tensor("xT", [D_MODEL, BLb], BF16, kind="ExternalInput").ap()
    # w* arrive host-pre-shuffled: [128, KT*cols] with k-tile k at cols
    # [k*cols, (k+1)*cols) — row p holds W[128k+p, :] pieces.
    wq = nc.dram_tensor("wq", [128, KT * MLOC], BF16, kind="ExternalInput").ap()
    wk = nc.dram_tensor("wk", [128, KT * MLOC], BF16, kind="ExternalInput").ap()
    wv = nc.dram_tensor("wv", [128, KT * MLOC], BF16, kind="ExternalInput").ap()
    wo = nc.dram_tensor("wo", [MLOC, D_MODEL], BF16, kind="ExternalInput").ap()
    bq = nc.dram_tensor("bq", [MLOC, 1], F32, kind="ExternalInput").ap()
    bk = nc.dram_tensor("bk", [MLOC, 1], F32, kind="ExternalInput").ap()
    bv = nc.dram_tensor("bv", [MLOC, 1], F32, kind="ExternalInput").ap()
    out = nc.dram_tensor("out", [BLb, D_MODEL], BF16, kind="ExternalOutput").ap()

    with tile.TileContext(nc) as tc, ExitStack() as ctx:
        consts = ctx.enter_context(tc.tile_pool(name="consts", bufs=1))
        qk_sb = ctx.enter_context(tc.tile_pool(name="qk_sb", bufs=1))
        xt_pool = ctx.enter_context(tc.tile_pool(name="xt", bufs=2))
        vt_pool = ctx.enter_context(tc.tile_pool(name="vt", bufs=2))
        # Unified PSUM: big pool (2-bank slots ×3) shared by scoresT / projs /
        # o-proj / v-transposes; av pool one 2-bank tile. Total 8 banks.
        big_ps = ctx.enter_context(tc.tile_pool(name="big_ps", bufs=3, space="PSUM"))
        av_ps = ctx.enter_context(tc.tile_pool(name="av_ps", bufs=1, space="PSUM"))
        exp_pool = ctx.enter_context(tc.tile_pool(name="expT", bufs=12))
        att_sb = ctx.enter_context(tc.tile_pool(name="att_sb", bufs=3))
        out_pool = ctx.enter_context(tc.tile_pool(name="out_sb", bufs=6))

        def load_xts(nc_i, eng=None):
            """One 3D-AP DMA for a whole projection chunk's x tiles."""
            csl = slice(nc_i * nch, (nc_i + 1) * nch)
            xt = xt_pool.tile([128, KT, nch], BF16, tag="xt", name="xt")
            src = xT.rearrange("(k p) c -> p k c", p=128)
            (eng or nc.sync).dma_start(xt[:], src[:, :, csl])
            return xt

        # Weights resident in SBUF: k-tile k of w* at [:, k, :]. Flat 2D
        # DMAs (host pre-shuffled). ScalarE's queue is idle until the first
        # exp; it carries the startup-critical x piece + wq so the PE can
        # start earliest.
        wq_sb = consts.tile([128, KT, MLOC], BF16, tag="wq")
        wk_sb = consts.tile([128, KT, MLOC], BF16, tag="wk")
        wv_sb = consts.tile([128, KT, MLOC], BF16, tag="wv")
        wo_sb = consts.tile([128, D_MODEL], BF16, tag="wo")
        ident = consts.tile([128, 128], BF16, tag="ident")
        masks.make_identity(nc, ident[:])  # gpsimd compute, fast
        nc.scalar.dma_start(wq_sb.rearrange("p k m -> p (k m)"), wq)
        xts_pf = xt_pool.tile([128, KT, nch], BF16, tag="xt", name="xt")
        xsrc = xT.rearrange("(k p) c -> p k c", p=128)
        for i, eng in ((0, nc.scalar), (1, nc.sync), (2, nc.gpsimd), (3, nc.sync)):
            eng.dma_start(xts_pf[:, 2 * i : 2 * i + 2, :],
                          xsrc[:, 2 * i : 2 * i + 2, 0:nch])
        nc.gpsimd.dma_start(wv_sb.rearrange("p k m -> p (k m)"), wv)
        nc.sync.dma_start(wk_sb.rearrange("p k m -> p (k m)"), wk)
        bq_sb = consts.tile([MLOC, 1], F32, tag="bq")
        bk_sb = consts.tile([MLOC, 1], F32, tag="bk")
        bv_sb = consts.tile([MLOC, 1], F32, tag="bv")
        for b_sb, b_dram in ((bv_sb, bv), (bq_sb, bq), (bk_sb, bk)):
            nc.gpsimd.dma_start(b_sb[:], b_dram)
        nc.gpsimd.dma_start(wo_sb[:], wo)
        ones_f32 = consts.tile([65, 128], F32, tag="ones_f32")
        nc.vector.memset(ones_f32[:], 1.0)

        # Persistent activations.
        qT_sb = qk_sb.tile([128, BLb], BF16, tag="qT")  # [d_local, b*Lb+l]
        kT_sb = qk_sb.tile([128, BLb], BF16, tag="kT")
        # v (natural layout) + ones columns, both heads in one tile:
        # per b: [128 s, n_st, 130] with head h at cols 65h..65h+64, ones at
        # col 65h+64 — so lhsT for attn@v head h is [:, st, 65h:65h+65].
        vaug = [qk_sb.tile([128, n_st, 2 * (HEAD_DIM + 1)], BF16,
                           tag=f"vaug{bi}", name=f"vaug{bi}")
                for bi in range(B)]
        for bi in range(B):
            for h in range(HPC):
                col = (HEAD_DIM + 1) * h + HEAD_DIM
                nc.vector.memset(vaug[bi][:, :, col : col + 1], 1.0)

        def proj_mm(nc_i, xts):
            """q/k/vT projections for one column chunk of x. Returns the vt
            SBUF tile whose transposes (proj_fin) are emitted later."""
            csl = slice(nc_i * nch, (nc_i + 1) * nch)
            ps_qk = big_ps.tile([128, 2, nch], F32, tag="big", name="ps_qk")
            for k in range(KT):
                nc.tensor.matmul(ps_qk[:, 0, :], wq_sb[:, k, :], xts[:, k, :],
                                 start=(k == 0), stop=(k == KT - 1))
                nc.tensor.matmul(ps_qk[:, 1, :], wk_sb[:, k, :], xts[:, k, :],
                                 start=(k == 0), stop=(k == KT - 1))
            # vT in the same contraction-major form (full N=512 streams).
            ps_v = big_ps.tile([128, nch], F32, tag="big", name="ps_v")
            for k in range(KT):
                nc.tensor.matmul(ps_v[:], wv_sb[:, k, :], xts[:, k, :],
                                 start=(k == 0), stop=(k == KT - 1))
            nc.vector.tensor_scalar_add(qT_sb[:, csl], ps_qk[:, 0, :], bq_sb[:])
            nc.vector.tensor_scalar_add(kT_sb[:, csl], ps_qk[:, 1, :], bk_sb[:])
            vt = vt_pool.tile([128, nch], BF16, tag="vt", name="vt")
            nc.vector.tensor_scalar_add(vt[:], ps_v[:], bv_sb[:])  # bv folded
            return vt

        def proj_fin(nc_i, vt):
            """PE-transpose vt [d, s] into vaug's [s, d] slots (both heads)."""
            ps_t = big_ps.tile([128, st_per_nch, 128], BF16, tag="big",
                               name="ps_t")
            for st in range(st_per_nch):
                nc.tensor.transpose(ps_t[:, st, :],
                                    vt[:, 128 * st : 128 * (st + 1)], ident[:])
            st_g = nc_i * st_per_nch
            bi, st_b = divmod(st_g, n_st)
            dst = vaug[bi][:, st_b : st_b + st_per_nch, :].rearrange(
                "p s (h d) -> p s h d", h=2, d=HEAD_DIM + 1)[:, :, :, :HEAD_DIM]
            src = ps_t.rearrange("p s (h d) -> p s h d", h=2, d=HEAD_DIM)
            nc.vector.tensor_copy(dst, src)

        def norm_part(avs_h, width):
            """Whole-chunk normalization chain -> oT (bf16 lhsT for o-proj),
            denominator broadcast on gpsimd."""
            den = att_sb.tile([1, 2, lc_size], F32, tag="den", name="den")
            rcp = att_sb.tile([128, 2, lc_size], F32, tag="rcp", name="rcp")
            for h in range(HPC):
                nc.vector.tensor_copy(den[0:1, h, :width], avs_h[h][64:65, :width])
            bden = att_sb.tile([128, 2, lc_size], F32, tag="bden", name="bden")
            nc.gpsimd.partition_broadcast(bden[:, :, :width],
                                          den[0:1, :, :width])
            nc.vector.reciprocal_approx_fast(rcp[:, :, :width],
                                             bden[:, :, :width])
            oT = att_sb.tile([128, lc_size], BF16, tag="oT", name="oT", bufs=6)
            for h in range(HPC):
                hsl = slice(64 * h, 64 * (h + 1))
                nc.vector.tensor_mul(oT[hsl, :width], avs_h[h][:HEAD_DIM, :width],
                                     rcp[:HEAD_DIM, h, :width])
            return oT[:, :width]

        def oproj_strip(oT, bi, loff, lt, use_act=False):
            """Output projection of one 128-row strip. use_act evacuates on
            ScalarE (activation Copy) — for the tail, where it is idle."""
            ps_o = big_ps.tile([128, 2, 512], F32, tag="big", name="ps_o")
            for dh in range(2):
                nc.tensor.matmul(ps_o[:, dh, :],
                                 oT[:, 128 * lt : 128 * (lt + 1)],
                                 wo_sb[:, 512 * dh : 512 * (dh + 1)],
                                 start=True, stop=True)
            ob = out_pool.tile([128, D_MODEL], BF16, tag="ob")
            if use_act:
                nc.scalar.copy(ob[:], ps_o.rearrange("p a b -> p (a b)"))
            else:
                nc.vector.tensor_copy(ob[:], ps_o.rearrange("p a b -> p (a b)"))
            # Tail stores split across the idle gpsimd queue + sync so the
            # final ~1MB drain runs on two DMA queues instead of one (the
            # last gpsimd broadcast is long done by then; mid-chunk stores
            # must stay off gpsimd or they delay the norm broadcasts).
            st_eng = nc.gpsimd if (use_act and lt % 2 == 0) else nc.sync
            st_eng.dma_start(
                out[bi * Lb + loff + 128 * lt : bi * Lb + loff + 128 * (lt + 1), :],
                ob[:])

        # Filler schedule: (chunk_idx, st) -> projection chunk to emit there.
        # proj c must land before scores need kT chunk c (sc(4(c%4)) of batch
        # c//4) and before av needs vaug (av trails sc by GLOBAL_LAG).
        filler_at = {(0, 2): 1, (0, 6): 2, (0, 10): 3,
                     (1, 4): 4, (2, 4): 5, (3, 4): 6, (4, 2): 7}

        chunks = []
        for bi in range(B):
            for lc in range(n_lc):
                chunks.append((bi, lc * lc_size, lc_size))
        n_ch = len(chunks)

        GLOBAL_LAG = 4  # av stream trails sc stream by this many units

        # Per-chunk state.
        exs = [[None] * n_st for _ in range(n_ch)]
        ps_avs = [None] * n_ch
        avs_done = [None] * n_ch  # evacuated avs SBUF tiles

        def do_sc(ci, st):
            bi, loff, width = chunks[ci]
            lsl = slice(bi * Lb + loff, bi * Lb + loff + width)
            ssl = slice(bi * Lb + st * 128, bi * Lb + (st + 1) * 128)
            ps_sc = big_ps.tile([128, HPC, lc_size], F32, tag="big",
                                name="ps_sc")
            for h in range(HPC):
                hsl = slice(64 * h, 64 * (h + 1))
                nc.tensor.matmul(ps_sc[:, h, :width], kT_sb[hsl, ssl],
                                 qT_sb[hsl, lsl],
                                 start=True, stop=True,
                                 tile_position=(64 * h, 0))
            ex = exp_pool.tile([128, HPC, lc_size], BF16, tag="ex", name="ex")
            nc.scalar.activation(ex[:, :, :width], ps_sc[:, :, :width],
                                 mybir.ActivationFunctionType.Exp,
                                 scale=1.0 / np.sqrt(HEAD_DIM))
            exs[ci][st] = ex

        def do_av(ci, st):
            bi, loff, width = chunks[ci]
            if st == 0:
                ps_avs[ci] = [av_ps.tile([HEAD_DIM + 1, lc_size], F32,
                                         tag=f"av{h}", name=f"av{h}")[:, :width]
                              for h in range(HPC)]
            for h in range(HPC):
                nc.tensor.matmul(ps_avs[ci][h][:],
                                 vaug[bi][:, st, 65 * h : 65 * h + 65],
                                 exs[ci][st][:, h, :width],
                                 start=(st == 0), stop=(st == n_st - 1))
            exs[ci][st] = None
            if st == n_st - 1:
                # Evacuate attn@v PSUM (frees the banks for the next chunk);
                # normalization is deferred to a later st-slot.
                avs = att_sb.tile([HEAD_DIM + 1, 2, lc_size], F32, tag="avs",
                                  name="avs")
                if ci == n_ch - 1:
                    for h in range(HPC):
                        nc.vector.tensor_copy(avs[64:65, h, :width],
                                              ps_avs[ci][h][64:65, :width])
                    for h in range(HPC):
                        nc.vector.tensor_copy(avs[:HEAD_DIM, h, :width],
                                              ps_avs[ci][h][:HEAD_DIM, :width])
                else:
                    for h in range(HPC):
                        nc.vector.tensor_copy(avs[:, h, :width],
                                              ps_avs[ci][h][:])
                avs_done[ci] = [avs[:, h, :] for h in range(HPC)]

        # Flat unit stream: sc(ci, st) in order; av trails by GLOBAL_LAG,
        # with the first av of each chunk held until st 8.
        units = [(ci, st) for ci in range(n_ch) for st in range(n_st)]
        pending_fin = None
        oproj_q = []
        av_ptr = 0
        xts_next = xts_pf
        next_load = 1
        for idx, (ci, st) in enumerate(units):
            if ci == 0 and st == 0:
                vt0 = proj_mm(0, xts_next)
                xts_next = load_xts(next_load)
                next_load += 1
                proj_fin(0, vt0)
            if pending_fin is not None:
                proj_fin(*pending_fin)
                pending_fin = None
            pj = filler_at.get((ci, st))
            if pj is not None:
                vt = proj_mm(pj, xts_next)
                if next_load < n_nch:
                    xts_next = load_xts(next_load)
                    next_load += 1
                pending_fin = (pj, vt)
            do_sc(ci, st)
            # Normalize the previous chunk once its avs are evacuated.
            if st == 5 and ci > 0:
                bi_p, loff_p, width_p = chunks[ci - 1]
                oproj_q.append((norm_part(avs_done[ci - 1], width_p),
                                bi_p, loff_p))
            # O-proj strips on odd s-tiles of the chunk's second half so each
            # PSUM slot has ~2us of PE work before ring reuse.
            if st >= n_st - 2 * n_lt + 1 and (st - n_st) % 2 == 1 and oproj_q:
                oproj_strip(*oproj_q[0], lt=(st - (n_st - 2 * n_lt + 1)) // 2)
                if st == n_st - 1:
                    oproj_q.pop(0)
            while av_ptr <= idx - GLOBAL_LAG:
                cj, stj = units[av_ptr]
                if stj == 0 and cj == ci and cj > 0 and st < 8:
                    break  # hold: let the previous chunk's evac clear first
                do_av(cj, stj)
                av_ptr += 1
        while av_ptr < len(units):
            do_av(*units[av_ptr])
            av_ptr += 1

        # Tail: last chunk's normalize + o-proj, strip-pipelined. The
        # denominator broadcast rides K=1 PE matmuls reading the avs row-64
        # slice directly (base partition 64 on both operands) and the ob
        # evacuations ride ScalarE — PE/ScalarE are otherwise idle here; DVE
        # keeps only the reciprocal and the oT scaling.
        bi_t, loff_t, width_t = chunks[-1]
        avs_h = avs_done[n_ch - 1]
        oT_t = att_sb.tile([128, lc_size], BF16, tag="oT", name="oT", bufs=6)
        ps_r = big_ps.tile([128, n_lt, 2, 128], F32, tag="big", name="ps_r")
        for lt in range(n_lt):
            for h in range(HPC):
                nc.tensor.matmul(ps_r[:, lt, h, :], ones_f32[64:65, :],
                                 avs_h[h][64:65, 128 * lt : 128 * (lt + 1)],
                                 start=True, stop=True)
        for lt in range(n_lt):
            wsl = slice(128 * lt, 128 * (lt + 1))
            rcp = att_sb.tile([128, 2, 128], F32, tag="rcps", name="rcps")
            nc.vector.reciprocal_approx_fast(rcp[:], ps_r[:, lt, :, :])
            for h in range(HPC):
                hsl = slice(64 * h, 64 * (h + 1))
                nc.vector.tensor_mul(oT_t[hsl, wsl], avs_h[h][:HEAD_DIM, wsl],
                                     rcp[:HEAD_DIM, h, :])
            oproj_strip(oT_t, bi_t, loff_t, lt, use_act=True)

    nc.compile()
    return nc


def make_in_maps(x, Wq, bq, Wk, bk, Wv, bv, Wo, Lb=L):
    """Per-core input dicts from full inputs. Weights are pre-shuffled to
    [128, KT*cols] (k-tile-major within rows) for clean 2D DMAs."""
    BLb = B * Lb
    KT = D_MODEL // 128
    xT = np.ascontiguousarray(
        np.asarray(x, np.float32).reshape(BLb, D_MODEL).T).astype(NPBF16)
    Wq = np.asarray(Wq, np.float32).astype(NPBF16)
    Wk = np.asarray(Wk, np.float32).astype(NPBF16)
    Wv = np.asarray(Wv, np.float32).astype(NPBF16)
    Wo = np.asarray(Wo, np.float32).astype(NPBF16)

    def shuffle_w(Wcol):  # [D_MODEL, MLOC] -> [128, KT*MLOC]
        return np.ascontiguousarray(
            Wcol.reshape(KT, 128, MLOC).transpose(1, 0, 2).reshape(128, KT * MLOC))

    in_maps = []
    for c in range(N_CORES):
        dsl = slice(MLOC * c, MLOC * (c + 1))
        in_maps.append({
            "xT": xT,
            "wq": shuffle_w(Wq[:, dsl]),
            "wk": shuffle_w(Wk[:, dsl]),
            "wv": shuffle_w(Wv[:, dsl]),
            "wo": np.ascontiguousarray(Wo[dsl, :]),
            "bq": np.ascontiguousarray(np.asarray(bq, np.float32)[dsl].reshape(MLOC, 1)),
            "bk": np.ascontiguousarray(np.asarray(bk, np.float32)[dsl].reshape(MLOC, 1)),
            "bv": np.ascontiguousarray(np.asarray(bv, np.float32)[dsl].reshape(MLOC, 1)),
        })
    return in_maps


_NC_CACHE = {}


def _get_nc():
    if "nc" not in _NC_CACHE:
        _NC_CACHE["nc"] = build_nc()
    return _NC_CACHE["nc"]


def kernel(x, Wq, bq, Wk, bk, Wv, bv, Wo, bo):
    nc = _get_nc()
    in_maps = make_in_maps(x, Wq, bq, Wk, bk, Wv, bv, Wo)
    res = run_bass_kernel_spmd(nc, in_maps, list(range(N_CORES)))
    acc = np.zeros((B * L, D_MODEL), dtype=np.float32)
    for c in range(N_CORES):
        acc += res.results[c]["out"]
    acc += np.asarray(bo, dtype=np.float32)
    return acc.reshape(B, L, D_MODEL)
